# revision 25
# baseline (speedup 1.0000x reference)
"""AdaptiveCentralLayer3D Trainium2 kernel.

Computes, per (b, c) channel of a (B, C, D, H, W)=(16, 64, 32, 32, 32) f32
tensor: the 3D centroid of the volume, integer shifts s = round(N/2 - c)
(round-half-even), then a cyclic roll of the volume by (sz, sy, sx) with
torch.roll semantics: out[i] = in[(i - s) mod N].

Strategy (8 NeuronCores, embarrassingly parallel over B*C; 128 volumes per
core, one volume per SBUF partition):

  Centroids (bit-exact vs the XLA-CPU oracle): XLA reduces sequentially in
  row-major order and TRN2's vector-engine tensor_reduce was verified to be
  bit-identical sequential f32 summation, so marginals S_d/S_h/S_w use
  access patterns matching XLA's reduce iteration order, ksum is one
  sequential reduce over all 32768 elements, the coordinate dot is
  elementwise mult + sequential reduce (verified to round identically to
  XLA's fma dot on the fixed inputs), and division is Newton-refined
  reciprocal (<=1 ulp; worst rounding margin on these inputs is 3 ulp).
  Round-half-even via the fp32 +/- 1.5*2^23 trick.

  Roll:
   - D and H rolls: an H-doubled staging copy in DRAM ([128, 32, 64, 32],
     written with two direct DMA passes), then one per-partition indirect
     gather per output d-plane: partition p reads 1024 contiguous f32 from
     element offset p*65536 + zd*2048 + ty*32.  (Only the [128,E]-dest +
     [128,1]-idx indirect-DMA form is used: the one form verified bit-exact
     on HW.)
   - W roll (within 128-byte rows, per-volume shift): on-chip sandwich:
     DVE 32x32 stream-transpose swaps (bc%32) with w so w lands on
     partitions, then 32 block-diagonal 0/1 permutation matmuls (one per
     bc%32 residue; fp32 matmuls with 0/1 weights are bit-exact) roll w,
     and a second stream-transpose restores bc to partitions in final
     (d, h, w) layout.
"""

import sys

if "/opt/trn_rl_repo" not in sys.path:
    sys.path.insert(0, "/opt/trn_rl_repo")

import numpy as np

import concourse.bass as bass
import concourse.mybir as mybir
import concourse.tile as tile
from concourse import bacc

F32 = mybir.dt.float32
I32 = mybir.dt.int32

BC = 128          # (b, c) volumes per core
D = H = W = 32
VOL = D * H * W       # 32768
PLANE2 = 2 * H * W    # 2048: one h-doubled plane
DBL = D * PLANE2      # 65536: h-doubled volume per bc
EPS = 1e-8
MAGIC = 12582912.0    # 1.5 * 2^23, fp32 round-to-nearest-even trick
NDC = 4               # d-planes per roll chunk
NCH = D // NDC        # 4 roll chunks
NDS = 4               # d-planes per staging write chunk


def build_kernel():
    nc = bacc.Bacc("TRN2", target_bir_lowering=False, debug=False)
    x = nc.dram_tensor("x", [BC, VOL], F32, kind="ExternalInput")
    out = nc.dram_tensor("out", [BC, VOL], F32, kind="ExternalOutput")

    with tile.TileContext(nc) as tc:
        with (
            tc.tile_pool(name="small", bufs=1) as small,
            tc.tile_pool(name="psum", bufs=2, space="PSUM") as psum_pool,
            tc.tile_pool(name="dram", bufs=1, space="DRAM") as dram_pool,
        ):
            stage = dram_pool.tile([BC, DBL], F32)
            # stage[p, d, hh, w] with hh in [0, 64): h-doubled planes
            stage4 = stage[:].rearrange("p (d q) -> p d q", q=PLANE2)

            with tc.tile_pool(name="bigA", bufs=1) as bigA:
                xin = bigA.tile([BC, VOL], F32)
                x4 = xin[:].rearrange("p (d h w) -> p d h w", h=H, w=W)
                s_d = small.tile([BC, D], F32)
                stage_writes = []
                NLC = 8  # d-planes per load chunk
                for ch in range(D // NLC):
                    nc.sync.dma_start(
                        xin[:, ch * NLC * H * W:(ch + 1) * NLC * H * W],
                        x[:, ch * NLC * H * W:(ch + 1) * NLC * H * W],
                    )
                    # per-chunk S_d (d-slices are independent)
                    nc.vector.tensor_reduce(
                        out=s_d[:, ch * NLC:(ch + 1) * NLC],
                        in_=x4[:, ch * NLC:(ch + 1) * NLC, :, :],
                        axis=mybir.AxisListType.XY, op=mybir.AluOpType.add,
                    )
                for ch in range(D // NDS):
                    sl = x4[:, ch * NDS:(ch + 1) * NDS, :, :]
                    st = stage4[:, ch * NDS:(ch + 1) * NDS, :].rearrange(
                        "p d (hh q) -> p d hh q", hh=2
                    )
                    eng = nc.sync if ch % 2 == 0 else nc.gpsimd
                    stage_writes.append(eng.dma_start(st[:, :, 0, :], sl))
                    eng2 = nc.gpsimd if ch % 2 == 0 else nc.sync
                    stage_writes.append(eng2.dma_start(st[:, :, 1, :], sl))

                # ---- centroids: bit-exact XLA-order reductions ----
                # ksum & marginals on ScalarE: activation-accumulate was
                # verified to be bit-identical sequential f32 summation in
                # the access pattern's iteration order.  A stride-0
                # broadcast dummy out avoids materializing func(x).
                adum = small.tile([BC, 1], F32)
                AF = mybir.ActivationFunctionType
                ksum = small.tile([BC, 1], F32)
                nc.scalar.activation(
                    adum[:, 0:1].to_broadcast([BC, VOL]), xin[:],
                    AF.Copy, accum_out=ksum[:],
                )
                nc.vector.tensor_scalar(
                    out=ksum[:], in0=ksum[:], scalar1=EPS, scalar2=None,
                    op0=mybir.AluOpType.add,
                )
                s_h = small.tile([BC, H], F32)
                nc.vector.tensor_reduce(
                    out=s_h[:],
                    in_=xin[:].rearrange("p (d h w) -> p h d w", h=H, w=W),
                    axis=mybir.AxisListType.XY, op=mybir.AluOpType.add,
                )

                coord_i = small.tile([BC, W], I32)
                nc.gpsimd.iota(coord_i[:], pattern=[[1, W]],
                               channel_multiplier=0)
                coord_f = small.tile([BC, W], F32)
                nc.vector.tensor_copy(coord_f[:], coord_i[:])

                r0 = small.tile([BC, 1], F32)
                nc.vector.reciprocal(r0[:], ksum[:])

                def centroid_offsets(s_list, tag):
                    """offv[:, i] = 128 - rint(16 - num_i/ksum), bit-exact."""
                    na = len(s_list)
                    nums = small.tile([BC, na], F32, tag=f"nums{tag}")
                    junk = small.tile([BC, W], F32, tag=f"junk{tag}")
                    for i, s_a in enumerate(s_list):
                        nc.vector.tensor_tensor(
                            out=junk[:], in0=s_a[:], in1=coord_f[:],
                            op=mybir.AluOpType.mult,
                        )
                        nc.vector.tensor_reduce(
                            out=nums[:, i:i + 1], in_=junk[:],
                            axis=mybir.AxisListType.X,
                            op=mybir.AluOpType.add,
                        )
                    r0b = r0[:].to_broadcast([BC, na])
                    ksb = ksum[:].to_broadcast([BC, na])
                    q0 = small.tile([BC, na], F32, tag=f"q0{tag}")
                    nc.vector.tensor_tensor(out=q0[:], in0=nums[:], in1=r0b,
                                            op=mybir.AluOpType.mult)
                    t1v = small.tile([BC, na], F32, tag=f"t1v{tag}")
                    nc.vector.tensor_tensor(out=t1v[:], in0=q0[:], in1=ksb,
                                            op=mybir.AluOpType.mult)
                    nc.vector.tensor_tensor(out=t1v[:], in0=nums[:],
                                            in1=t1v[:],
                                            op=mybir.AluOpType.subtract)
                    nc.vector.tensor_tensor(out=t1v[:], in0=t1v[:], in1=r0b,
                                            op=mybir.AluOpType.mult)
                    cv = small.tile([BC, na], F32, tag=f"cv{tag}")
                    nc.vector.tensor_tensor(out=cv[:], in0=q0[:], in1=t1v[:],
                                            op=mybir.AluOpType.add)
                    nc.vector.tensor_scalar(
                        out=cv[:], in0=cv[:], scalar1=-1.0,
                        scalar2=float(D // 2), op0=mybir.AluOpType.mult,
                        op1=mybir.AluOpType.add,
                    )
                    nc.vector.tensor_scalar(
                        out=cv[:], in0=cv[:], scalar1=MAGIC, scalar2=MAGIC,
                        op0=mybir.AluOpType.add,
                        op1=mybir.AluOpType.subtract,
                    )
                    off = small.tile([BC, na], F32, tag=f"off{tag}")
                    nc.vector.tensor_scalar(
                        out=off[:], in0=cv[:], scalar1=-1.0, scalar2=128.0,
                        op0=mybir.AluOpType.mult, op1=mybir.AluOpType.add,
                    )
                    return off

                offzy = centroid_offsets((s_d, s_h), "zy")

                # gather offsets: idx[p, d] = p*DBL + zd*PLANE2 + ty*W
                zd_i = small.tile([BC, D], I32)
                nc.gpsimd.iota(zd_i[:], pattern=[[1, D]],
                               channel_multiplier=0)
                offz_i = small.tile([BC, 1], I32)
                nc.vector.tensor_copy(offz_i[:], offzy[:, 0:1])
                nc.vector.tensor_tensor(
                    out=zd_i[:], in0=zd_i[:],
                    in1=offz_i[:].to_broadcast([BC, D]),
                    op=mybir.AluOpType.add,
                )
                nc.vector.tensor_scalar(
                    out=zd_i[:], in0=zd_i[:], scalar1=31, scalar2=None,
                    op0=mybir.AluOpType.bitwise_and,
                )
                idx_f = small.tile([BC, D], F32)
                nc.vector.tensor_copy(idx_f[:], zd_i[:])
                nc.vector.tensor_scalar(
                    out=idx_f[:], in0=idx_f[:], scalar1=float(PLANE2 // W),
                    scalar2=None, op0=mybir.AluOpType.mult,
                )
                ty_i = small.tile([BC, 1], I32)
                nc.vector.tensor_copy(ty_i[:], offzy[:, 1:2])
                nc.vector.tensor_scalar(
                    out=ty_i[:], in0=ty_i[:], scalar1=31, scalar2=None,
                    op0=mybir.AluOpType.bitwise_and,
                )
                ty_f = small.tile([BC, 1], F32)
                nc.vector.tensor_copy(ty_f[:], ty_i[:])
                nc.vector.tensor_scalar(
                    out=idx_f[:], in0=idx_f[:], scalar1=ty_f[:], scalar2=None,
                    op0=mybir.AluOpType.add,
                )
                idx = small.tile([BC, D], I32)
                nc.vector.tensor_copy(idx[:], idx_f[:])
                base_i = small.tile([BC, 1], I32)
                nc.gpsimd.iota(base_i[:], pattern=[[0, 1]],
                               channel_multiplier=DBL // W)
                idx_done = nc.vector.tensor_tensor(
                    out=idx[:], in0=idx[:],
                    in1=base_i[:].to_broadcast([BC, D]),
                    op=mybir.AluOpType.add,
                )

                s_w = small.tile([BC, W], F32)
                sw_red = nc.vector.tensor_reduce(
                    out=s_w[:],
                    in_=xin[:].rearrange("p (d h w) -> p w d h", h=H, w=W),
                    axis=mybir.AxisListType.XY, op=mybir.AluOpType.add,
                )
                from concourse.tile_rust import add_dep_helper as _adh
                _adh(sw_red.ins, idx_done.ins,
                     reason="defer S_w behind the gather index chain")
                offx = centroid_offsets((s_w,), "x")

                # ---- W-roll permutation stationaries SW[:, r, :] ----
                # SW_r[(pb,w_in), (pb',w_out)] =
                #   [pb==pb'] * [w_in == (w_out + tx[32*pb'+r]) & 31]
                tx_i = small.tile([BC, 1], I32)
                nc.vector.tensor_copy(tx_i[:], offx[:, 0:1])
                nc.vector.tensor_scalar(
                    out=tx_i[:], in0=tx_i[:], scalar1=31, scalar2=None,
                    op0=mybir.AluOpType.bitwise_and,
                )
                tx_f = small.tile([BC, 1], F32)
                nc.vector.tensor_copy(tx_f[:], tx_i[:])
                # transpose tx [128,1] -> row [1,128] via DMA, then replicate
                # to all partitions with a K=1 ones matmul
                tx_dram = dram_pool.tile([BC, 1], F32)
                nc.sync.dma_start(tx_dram[:], tx_f[:])
                txT = small.tile([1, BC], F32)
                nc.sync.dma_start(
                    txT[0:1, :], tx_dram[:].rearrange("p o -> o p")
                )
                ones1 = small.tile([1, BC], F32)
                nc.vector.memset(ones1[:], 1.0)
                txRep_ps = psum_pool.tile([BC, BC], F32)
                nc.tensor.matmul(txRep_ps[:], lhsT=ones1[:], rhs=txT[:],
                                 start=True, stop=True)
                txRep = small.tile([BC, BC], F32)
                nc.scalar.copy(txRep[:], txRep_ps[:])

                # per-partition row scalars: w_in = p & 31, pb32 = p & 96
                rowi = small.tile([BC, 1], I32)
                nc.gpsimd.iota(rowi[:], pattern=[[0, 1]],
                               channel_multiplier=1)
                rowW = small.tile([BC, 1], F32)
                roww_i = small.tile([BC, 1], I32)
                nc.vector.tensor_scalar(
                    out=roww_i[:], in0=rowi[:], scalar1=31, scalar2=None,
                    op0=mybir.AluOpType.bitwise_and,
                )
                nc.vector.tensor_copy(rowW[:], roww_i[:])
                rowPB = small.tile([BC, 1], F32)
                rowpb_i = small.tile([BC, 1], I32)
                nc.vector.tensor_scalar(
                    out=rowpb_i[:], in0=rowi[:], scalar1=96, scalar2=None,
                    op0=mybir.AluOpType.bitwise_and,
                )
                nc.vector.tensor_copy(rowPB[:], rowpb_i[:])

                # column patterns (same on every partition)
                iwo_i = small.tile([BC, BC], I32)   # w_out(m) = m & 31
                nc.gpsimd.iota(iwo_i[:], pattern=[[0, 4], [1, W]],
                               channel_multiplier=0)
                iwo = small.tile([BC, BC], F32)
                nc.vector.tensor_copy(iwo[:], iwo_i[:])
                cpb_i = small.tile([BC, BC], I32)   # pb32(m) = (m>>5)*32
                nc.gpsimd.iota(cpb_i[:], pattern=[[W, 4], [0, W]],
                               channel_multiplier=0)
                cpb = small.tile([BC, BC], F32)
                nc.vector.tensor_copy(cpb[:], cpb_i[:])
                m2 = small.tile([BC, BC], F32)      # [pb == pb']
                nc.vector.tensor_scalar(
                    out=m2[:], in0=cpb[:], scalar1=rowPB[:], scalar2=None,
                    op0=mybir.AluOpType.is_equal,
                )

                # A[p, r, m] = (w_out(m) + tx[32*(m>>5)+r]) mod 32
                sw = small.tile([BC, D, BC], F32)   # 16KB/partition
                txv = txRep[:].rearrange(
                    "p (pb r) -> p r pb", r=D
                ).unsqueeze(3).to_broadcast([BC, D, 4, W])
                iwb = iwo[:].rearrange(
                    "p (pb w) -> p pb w", w=W
                ).unsqueeze(1).to_broadcast([BC, D, 4, W])
                nc.vector.tensor_tensor(
                    out=sw[:].rearrange("p r (pb w) -> p r pb w", w=W),
                    in0=iwb, in1=txv, op=mybir.AluOpType.add,
                )
                amask = small.tile([BC, D, BC], F32)
                nc.vector.tensor_scalar(
                    out=amask[:], in0=sw[:], scalar1=31.5, scalar2=-32.0,
                    op0=mybir.AluOpType.is_gt, op1=mybir.AluOpType.mult,
                )
                nc.vector.tensor_tensor(
                    out=sw[:], in0=sw[:], in1=amask[:],
                    op=mybir.AluOpType.add,
                )
                # sw = [w_in == A] * [pb == pb']
                nc.vector.tensor_scalar(
                    out=sw[:], in0=sw[:], scalar1=rowW[:], scalar2=None,
                    op0=mybir.AluOpType.is_equal,
                )
                nc.vector.tensor_tensor(
                    out=sw[:], in0=sw[:],
                    in1=m2[:].unsqueeze(1).to_broadcast([BC, D, BC]),
                    op=mybir.AluOpType.mult,
                )

            # ---- roll: gather chunks, W-sandwich, store ----
            # Tile does not track the DRAM-tile dependency between the
            # rearranged stage-write views and the flattened gather view;
            # fence the gathers behind all stage writes explicitly.
            from concourse.tile_rust import add_dep_helper

            fence = nc.gpsimd.nop(nofuse=True, hint="stage_fence")
            for wi in stage_writes:
                add_dep_helper(fence.ins, wi.ins,
                               reason="gathers wait on staging writes")
            stage_flat = stage[:].rearrange("p (r w) -> (p r) w", w=W)
            with tc.tile_pool(name="bigC", bufs=1) as bigC:
                for ch in range(NCH):
                    g = bigC.tile([BC, NDC, H, W], F32, tag="G", bufs=2)
                    for j in range(NDC):
                        d_out = ch * NDC + j
                        gi = nc.gpsimd.indirect_dma_start(
                            out=g[:, j, :, :].rearrange("p h w -> p (h w)"),
                            out_offset=None,
                            in_=stage_flat,
                            in_offset=bass.IndirectOffsetOnAxis(
                                ap=idx[:, d_out:d_out + 1], axis=0,
                            ),
                        )
                        add_dep_helper(gi.ins, fence.ins,
                                       reason="gather waits on stage fence")
                    # T1: swap (bc%32) <-> w per 32x32 square (in-place grid)
                    t1o = bigC.tile([BC, NDC * H, W], F32, tag="T1", bufs=2)
                    nc.vector.transpose(
                        t1o[:].rearrange("p a b -> p (a b)"),
                        g[:].rearrange("p a h w -> p (a h w)"),
                    )
                    # 32 permutation matmuls: roll w (on partitions)
                    w2 = bigC.tile([BC, NDC * H, W], F32, tag="W2", bufs=2)
                    for rq in range(D // 4):
                        pst = psum_pool.tile([BC, 4, NDC * H], F32,
                                             tag="pmm")
                        for jj in range(4):
                            r = rq * 4 + jj
                            nc.tensor.matmul(
                                pst[:, jj, :],
                                lhsT=sw[:, r, :],
                                rhs=t1o[:, :, r],
                                start=True, stop=True,
                            )
                        nc.scalar.copy(
                            w2[:, :, rq * 4:(rq + 1) * 4],
                            pst[:].rearrange("p j n -> p n j"),
                        )
                    # T2: swap w <-> (bc%32) back; final (d, h, w) layout
                    fin = bigC.tile([BC, NDC * H * W], F32, tag="FIN", bufs=2)
                    nc.vector.transpose(
                        fin[:],
                        w2[:].rearrange("p a b -> p (a b)"),
                    )
                    nc.sync.dma_start(
                        out[:, ch * NDC * H * W:(ch + 1) * NDC * H * W],
                        fin[:],
                    )

    nc.compile()
    return nc


_NC_CACHE = None


def _get_nc():
    global _NC_CACHE
    if _NC_CACHE is None:
        _NC_CACHE = build_kernel()
    return _NC_CACHE


def _shard(inputs):
    k = np.ascontiguousarray(np.asarray(inputs["kernel"], dtype=np.float32))
    B, C, d, h, w = k.shape
    flat = k.reshape(B * C, d * h * w)
    per = flat.shape[0] // 8
    return k.shape, [{"x": flat[i * per:(i + 1) * per]} for i in range(8)]


def kernel(**inputs: np.ndarray) -> np.ndarray:
    from concourse.bass_utils import run_bass_kernel_spmd

    shape, in_maps = _shard(inputs)
    nc = _get_nc()
    res = run_bass_kernel_spmd(nc, in_maps, core_ids=list(range(8)))
    outs = [res.results[i]["out"] for i in range(8)]
    return np.concatenate(outs, axis=0).reshape(shape)


def profile_once(inputs):
    """Run once with NTFF tracing; return exec_time_ns or None."""
    from concourse.bass_utils import run_bass_kernel_spmd

    _, in_maps = _shard(inputs)
    try:
        res = run_bass_kernel_spmd(
            _get_nc(), in_maps, core_ids=list(range(8)), trace=True
        )
        return res.exec_time_ns
    except Exception as e:
        print(f"profile_once failed: {type(e).__name__}: {e}")
        return None


def _np_reference(xs):
    v = xs.reshape(BC, D, H, W).astype(np.float64)
    ks = v.sum(axis=(1, 2, 3)) + EPS
    z = np.arange(D)
    cz = np.einsum("pdhw,d->p", v, z) / ks
    cy = np.einsum("pdhw,h->p", v, z) / ks
    cx = np.einsum("pdhw,w->p", v, z) / ks
    sz, sy, sx = (np.round(D / 2 - c).astype(int) for c in (cz, cy, cx))
    exp = np.empty_like(v, dtype=np.float32)
    for p in range(BC):
        exp[p] = np.roll(
            v[p].astype(np.float32), (sz[p], sy[p], sx[p]), axis=(0, 1, 2)
        )
    return exp.reshape(BC, VOL)


if __name__ == "__main__":
    from concourse.bass_interp import CoreSim

    rng = np.random.default_rng(0)
    xs = rng.random((BC, VOL), dtype=np.float32)

    nc = build_kernel()
    sim = CoreSim(nc, trace=False)
    sim.tensor("x")[:] = xs
    sim.simulate()
    got = np.array(sim.tensor("out"))

    exp = _np_reference(xs)
    err = np.abs(got - exp)
    rel = np.linalg.norm(got - exp) / np.linalg.norm(exp)
    badvol = int((err.reshape(BC, -1).max(1) > 1e-5).sum())
    print("cost-model time:", sim._sim_state.time, "ns")
    print("max abs err:", err.max(), "rel:", rel)
    print("mismatched volumes (rounding-boundary flips are OK in sim):",
          badvol, "/", BC)
    assert badvol <= 3, "sim mismatch beyond rounding-boundary noise"
    print("CoreSim PASS")


# revision 26
# speedup vs baseline: 1.0470x; 1.0470x over previous
"""AdaptiveCentralLayer3D Trainium2 kernel.

Computes, per (b, c) channel of a (B, C, D, H, W)=(16, 64, 32, 32, 32) f32
tensor: the 3D centroid of the volume, integer shifts s = round(N/2 - c)
(round-half-even), then a cyclic roll of the volume by (sz, sy, sx) with
torch.roll semantics: out[i] = in[(i - s) mod N].

Strategy (8 NeuronCores, embarrassingly parallel over B*C; 128 volumes per
core, one volume per SBUF partition):

  Centroids (bit-exact vs the XLA-CPU oracle): XLA reduces sequentially in
  row-major order and TRN2's vector-engine tensor_reduce was verified to be
  bit-identical sequential f32 summation, so marginals S_d/S_h/S_w use
  access patterns matching XLA's reduce iteration order, ksum is one
  sequential reduce over all 32768 elements, the coordinate dot is
  elementwise mult + sequential reduce (verified to round identically to
  XLA's fma dot on the fixed inputs), and division is Newton-refined
  reciprocal (<=1 ulp; worst rounding margin on these inputs is 3 ulp).
  Round-half-even via the fp32 +/- 1.5*2^23 trick.

  Roll:
   - D and H rolls: an H-doubled staging copy in DRAM ([128, 32, 64, 32],
     written with two direct DMA passes), then one per-partition indirect
     gather per output d-plane: partition p reads 1024 contiguous f32 from
     element offset p*65536 + zd*2048 + ty*32.  (Only the [128,E]-dest +
     [128,1]-idx indirect-DMA form is used: the one form verified bit-exact
     on HW.)
   - W roll (within 128-byte rows, per-volume shift): on-chip sandwich:
     DVE 32x32 stream-transpose swaps (bc%32) with w so w lands on
     partitions, then 32 block-diagonal 0/1 permutation matmuls (one per
     bc%32 residue; fp32 matmuls with 0/1 weights are bit-exact) roll w,
     and a second stream-transpose restores bc to partitions in final
     (d, h, w) layout.
"""

import sys

if "/opt/trn_rl_repo" not in sys.path:
    sys.path.insert(0, "/opt/trn_rl_repo")

import numpy as np

import concourse.bass as bass
import concourse.mybir as mybir
import concourse.tile as tile
from concourse import bacc

F32 = mybir.dt.float32
I32 = mybir.dt.int32

BC = 128          # (b, c) volumes per core
D = H = W = 32
VOL = D * H * W       # 32768
PLANE2 = 2 * H * W    # 2048: one h-doubled plane
DBL = D * PLANE2      # 65536: h-doubled volume per bc
EPS = 1e-8
MAGIC = 12582912.0    # 1.5 * 2^23, fp32 round-to-nearest-even trick
NDC = 4               # d-planes per roll chunk
NCH = D // NDC        # 4 roll chunks
NDS = 4               # d-planes per staging write chunk


def build_kernel():
    nc = bacc.Bacc("TRN2", target_bir_lowering=False, debug=False)
    x = nc.dram_tensor("x", [BC, VOL], F32, kind="ExternalInput")
    out = nc.dram_tensor("out", [BC, VOL], F32, kind="ExternalOutput")

    with tile.TileContext(nc) as tc:
        with (
            tc.tile_pool(name="small", bufs=1) as small,
            tc.tile_pool(name="psum", bufs=2, space="PSUM") as psum_pool,
            tc.tile_pool(name="dram", bufs=1, space="DRAM") as dram_pool,
        ):
            stage = dram_pool.tile([BC, DBL], F32)
            # stage[p, d, hh, w] with hh in [0, 64): h-doubled planes
            stage4 = stage[:].rearrange("p (d q) -> p d q", q=PLANE2)

            with tc.tile_pool(name="bigA", bufs=1) as bigA:
                xin = bigA.tile([BC, VOL], F32)
                x4 = xin[:].rearrange("p (d h w) -> p d h w", h=H, w=W)
                s_d = small.tile([BC, D], F32)
                stage_writes = []
                NLC = 8  # d-planes per load chunk
                for ch in range(D // NLC):
                    leng = nc.sync if ch % 2 == 0 else nc.gpsimd
                    leng.dma_start(
                        xin[:, ch * NLC * H * W:(ch + 1) * NLC * H * W],
                        x[:, ch * NLC * H * W:(ch + 1) * NLC * H * W],
                    )
                    # per-chunk S_d (d-slices are independent)
                    nc.vector.tensor_reduce(
                        out=s_d[:, ch * NLC:(ch + 1) * NLC],
                        in_=x4[:, ch * NLC:(ch + 1) * NLC, :, :],
                        axis=mybir.AxisListType.XY, op=mybir.AluOpType.add,
                    )
                for ch in range(D // NDS):
                    sl = x4[:, ch * NDS:(ch + 1) * NDS, :, :]
                    st = stage4[:, ch * NDS:(ch + 1) * NDS, :].rearrange(
                        "p d (hh q) -> p d hh q", hh=2
                    )
                    eng = nc.sync if ch % 2 == 0 else nc.gpsimd
                    stage_writes.append(eng.dma_start(st[:, :, 0, :], sl))
                    eng2 = nc.gpsimd if ch % 2 == 0 else nc.sync
                    stage_writes.append(eng2.dma_start(st[:, :, 1, :], sl))

                # ---- centroids: bit-exact XLA-order reductions ----
                # ksum & marginals on ScalarE: activation-accumulate was
                # verified to be bit-identical sequential f32 summation in
                # the access pattern's iteration order.  A stride-0
                # broadcast dummy out avoids materializing func(x).
                adum = small.tile([BC, 1], F32)
                AF = mybir.ActivationFunctionType
                ksum = small.tile([BC, 1], F32)
                nc.scalar.activation(
                    adum[:, 0:1].to_broadcast([BC, VOL]), xin[:],
                    AF.Copy, accum_out=ksum[:],
                )
                nc.vector.tensor_scalar(
                    out=ksum[:], in0=ksum[:], scalar1=EPS, scalar2=None,
                    op0=mybir.AluOpType.add,
                )
                s_h = small.tile([BC, H], F32)
                nc.vector.tensor_reduce(
                    out=s_h[:],
                    in_=xin[:].rearrange("p (d h w) -> p h d w", h=H, w=W),
                    axis=mybir.AxisListType.XY, op=mybir.AluOpType.add,
                )

                coord_i = small.tile([BC, W], I32)
                nc.gpsimd.iota(coord_i[:], pattern=[[1, W]],
                               channel_multiplier=0)
                coord_f = small.tile([BC, W], F32)
                nc.vector.tensor_copy(coord_f[:], coord_i[:])

                r0 = small.tile([BC, 1], F32)
                nc.vector.reciprocal(r0[:], ksum[:])

                def centroid_offsets(s_list, tag):
                    """offv[:, i] = 128 - rint(16 - num_i/ksum), bit-exact."""
                    na = len(s_list)
                    nums = small.tile([BC, na], F32, tag=f"nums{tag}")
                    junk = small.tile([BC, W], F32, tag=f"junk{tag}")
                    for i, s_a in enumerate(s_list):
                        nc.vector.tensor_tensor(
                            out=junk[:], in0=s_a[:], in1=coord_f[:],
                            op=mybir.AluOpType.mult,
                        )
                        nc.vector.tensor_reduce(
                            out=nums[:, i:i + 1], in_=junk[:],
                            axis=mybir.AxisListType.X,
                            op=mybir.AluOpType.add,
                        )
                    r0b = r0[:].to_broadcast([BC, na])
                    ksb = ksum[:].to_broadcast([BC, na])
                    q0 = small.tile([BC, na], F32, tag=f"q0{tag}")
                    nc.vector.tensor_tensor(out=q0[:], in0=nums[:], in1=r0b,
                                            op=mybir.AluOpType.mult)
                    t1v = small.tile([BC, na], F32, tag=f"t1v{tag}")
                    nc.vector.tensor_tensor(out=t1v[:], in0=q0[:], in1=ksb,
                                            op=mybir.AluOpType.mult)
                    nc.vector.tensor_tensor(out=t1v[:], in0=nums[:],
                                            in1=t1v[:],
                                            op=mybir.AluOpType.subtract)
                    nc.vector.tensor_tensor(out=t1v[:], in0=t1v[:], in1=r0b,
                                            op=mybir.AluOpType.mult)
                    cv = small.tile([BC, na], F32, tag=f"cv{tag}")
                    nc.vector.tensor_tensor(out=cv[:], in0=q0[:], in1=t1v[:],
                                            op=mybir.AluOpType.add)
                    nc.vector.tensor_scalar(
                        out=cv[:], in0=cv[:], scalar1=-1.0,
                        scalar2=float(D // 2), op0=mybir.AluOpType.mult,
                        op1=mybir.AluOpType.add,
                    )
                    nc.vector.tensor_scalar(
                        out=cv[:], in0=cv[:], scalar1=MAGIC, scalar2=MAGIC,
                        op0=mybir.AluOpType.add,
                        op1=mybir.AluOpType.subtract,
                    )
                    off = small.tile([BC, na], F32, tag=f"off{tag}")
                    nc.vector.tensor_scalar(
                        out=off[:], in0=cv[:], scalar1=-1.0, scalar2=128.0,
                        op0=mybir.AluOpType.mult, op1=mybir.AluOpType.add,
                    )
                    return off

                offzy = centroid_offsets((s_d, s_h), "zy")

                # gather offsets: idx[p, d] = p*DBL + zd*PLANE2 + ty*W
                zd_i = small.tile([BC, D], I32)
                nc.gpsimd.iota(zd_i[:], pattern=[[1, D]],
                               channel_multiplier=0)
                offz_i = small.tile([BC, 1], I32)
                nc.vector.tensor_copy(offz_i[:], offzy[:, 0:1])
                nc.vector.tensor_tensor(
                    out=zd_i[:], in0=zd_i[:],
                    in1=offz_i[:].to_broadcast([BC, D]),
                    op=mybir.AluOpType.add,
                )
                nc.vector.tensor_scalar(
                    out=zd_i[:], in0=zd_i[:], scalar1=31, scalar2=None,
                    op0=mybir.AluOpType.bitwise_and,
                )
                idx_f = small.tile([BC, D], F32)
                nc.vector.tensor_copy(idx_f[:], zd_i[:])
                nc.vector.tensor_scalar(
                    out=idx_f[:], in0=idx_f[:], scalar1=float(PLANE2 // W),
                    scalar2=None, op0=mybir.AluOpType.mult,
                )
                ty_i = small.tile([BC, 1], I32)
                nc.vector.tensor_copy(ty_i[:], offzy[:, 1:2])
                nc.vector.tensor_scalar(
                    out=ty_i[:], in0=ty_i[:], scalar1=31, scalar2=None,
                    op0=mybir.AluOpType.bitwise_and,
                )
                ty_f = small.tile([BC, 1], F32)
                nc.vector.tensor_copy(ty_f[:], ty_i[:])
                nc.vector.tensor_scalar(
                    out=idx_f[:], in0=idx_f[:], scalar1=ty_f[:], scalar2=None,
                    op0=mybir.AluOpType.add,
                )
                idx = small.tile([BC, D], I32)
                nc.vector.tensor_copy(idx[:], idx_f[:])
                base_i = small.tile([BC, 1], I32)
                nc.gpsimd.iota(base_i[:], pattern=[[0, 1]],
                               channel_multiplier=DBL // W)
                idx_done = nc.vector.tensor_tensor(
                    out=idx[:], in0=idx[:],
                    in1=base_i[:].to_broadcast([BC, D]),
                    op=mybir.AluOpType.add,
                )

                s_w = small.tile([BC, W], F32)
                sw_red = nc.vector.tensor_reduce(
                    out=s_w[:],
                    in_=xin[:].rearrange("p (d h w) -> p w d h", h=H, w=W),
                    axis=mybir.AxisListType.XY, op=mybir.AluOpType.add,
                )
                from concourse.tile_rust import add_dep_helper as _adh
                _adh(sw_red.ins, idx_done.ins,
                     reason="defer S_w behind the gather index chain")
                offx = centroid_offsets((s_w,), "x")

                # ---- W-roll permutation stationaries SW[:, r, :] ----
                # SW_r[(pb,w_in), (pb',w_out)] =
                #   [pb==pb'] * [w_in == (w_out + tx[32*pb'+r]) & 31]
                tx_i = small.tile([BC, 1], I32)
                nc.vector.tensor_copy(tx_i[:], offx[:, 0:1])
                nc.vector.tensor_scalar(
                    out=tx_i[:], in0=tx_i[:], scalar1=31, scalar2=None,
                    op0=mybir.AluOpType.bitwise_and,
                )
                tx_f = small.tile([BC, 1], F32)
                nc.vector.tensor_copy(tx_f[:], tx_i[:])
                # transpose tx [128,1] -> row [1,128] via DMA, then replicate
                # to all partitions with a K=1 ones matmul
                tx_dram = dram_pool.tile([BC, 1], F32)
                nc.sync.dma_start(tx_dram[:], tx_f[:])
                txT = small.tile([1, BC], F32)
                nc.sync.dma_start(
                    txT[0:1, :], tx_dram[:].rearrange("p o -> o p")
                )
                ones1 = small.tile([1, BC], F32)
                nc.vector.memset(ones1[:], 1.0)
                txRep_ps = psum_pool.tile([BC, BC], F32)
                nc.tensor.matmul(txRep_ps[:], lhsT=ones1[:], rhs=txT[:],
                                 start=True, stop=True)
                txRep = small.tile([BC, BC], F32)
                nc.scalar.copy(txRep[:], txRep_ps[:])

                # per-partition row scalars: w_in = p & 31, pb32 = p & 96
                rowi = small.tile([BC, 1], I32)
                nc.gpsimd.iota(rowi[:], pattern=[[0, 1]],
                               channel_multiplier=1)
                rowW = small.tile([BC, 1], F32)
                roww_i = small.tile([BC, 1], I32)
                nc.vector.tensor_scalar(
                    out=roww_i[:], in0=rowi[:], scalar1=31, scalar2=None,
                    op0=mybir.AluOpType.bitwise_and,
                )
                nc.vector.tensor_copy(rowW[:], roww_i[:])
                rowPB = small.tile([BC, 1], F32)
                rowpb_i = small.tile([BC, 1], I32)
                nc.vector.tensor_scalar(
                    out=rowpb_i[:], in0=rowi[:], scalar1=96, scalar2=None,
                    op0=mybir.AluOpType.bitwise_and,
                )
                nc.vector.tensor_copy(rowPB[:], rowpb_i[:])

                # column patterns (same on every partition)
                iwo_i = small.tile([BC, BC], I32)   # w_out(m) = m & 31
                nc.gpsimd.iota(iwo_i[:], pattern=[[0, 4], [1, W]],
                               channel_multiplier=0)
                iwo = small.tile([BC, BC], F32)
                nc.vector.tensor_copy(iwo[:], iwo_i[:])
                cpb_i = small.tile([BC, BC], I32)   # pb32(m) = (m>>5)*32
                nc.gpsimd.iota(cpb_i[:], pattern=[[W, 4], [0, W]],
                               channel_multiplier=0)
                cpb = small.tile([BC, BC], F32)
                nc.vector.tensor_copy(cpb[:], cpb_i[:])
                m2 = small.tile([BC, BC], F32)      # [pb == pb']
                nc.vector.tensor_scalar(
                    out=m2[:], in0=cpb[:], scalar1=rowPB[:], scalar2=None,
                    op0=mybir.AluOpType.is_equal,
                )

                # A[p, r, m] = (w_out(m) + tx[32*(m>>5)+r]) mod 32
                sw = small.tile([BC, D, BC], F32)   # 16KB/partition
                txv = txRep[:].rearrange(
                    "p (pb r) -> p r pb", r=D
                ).unsqueeze(3).to_broadcast([BC, D, 4, W])
                iwb = iwo[:].rearrange(
                    "p (pb w) -> p pb w", w=W
                ).unsqueeze(1).to_broadcast([BC, D, 4, W])
                nc.vector.tensor_tensor(
                    out=sw[:].rearrange("p r (pb w) -> p r pb w", w=W),
                    in0=iwb, in1=txv, op=mybir.AluOpType.add,
                )
                amask = small.tile([BC, D, BC], F32)
                nc.vector.tensor_scalar(
                    out=amask[:], in0=sw[:], scalar1=31.5, scalar2=-32.0,
                    op0=mybir.AluOpType.is_gt, op1=mybir.AluOpType.mult,
                )
                nc.vector.tensor_tensor(
                    out=sw[:], in0=sw[:], in1=amask[:],
                    op=mybir.AluOpType.add,
                )
                # sw = [w_in == A] * [pb == pb']
                nc.vector.tensor_scalar(
                    out=sw[:], in0=sw[:], scalar1=rowW[:], scalar2=None,
                    op0=mybir.AluOpType.is_equal,
                )
                nc.vector.tensor_tensor(
                    out=sw[:], in0=sw[:],
                    in1=m2[:].unsqueeze(1).to_broadcast([BC, D, BC]),
                    op=mybir.AluOpType.mult,
                )

            # ---- roll: gather chunks, W-sandwich, store ----
            # Tile does not track the DRAM-tile dependency between the
            # rearranged stage-write views and the flattened gather view;
            # fence the gathers behind all stage writes explicitly.
            from concourse.tile_rust import add_dep_helper

            fence = nc.gpsimd.nop(nofuse=True, hint="stage_fence")
            for wi in stage_writes:
                add_dep_helper(fence.ins, wi.ins,
                               reason="gathers wait on staging writes")
            stage_flat = stage[:].rearrange("p (r w) -> (p r) w", w=W)
            with tc.tile_pool(name="bigC", bufs=1) as bigC:
                for ch in range(NCH):
                    g = bigC.tile([BC, NDC, H, W], F32, tag="G", bufs=2)
                    for j in range(NDC):
                        d_out = ch * NDC + j
                        gi = nc.gpsimd.indirect_dma_start(
                            out=g[:, j, :, :].rearrange("p h w -> p (h w)"),
                            out_offset=None,
                            in_=stage_flat,
                            in_offset=bass.IndirectOffsetOnAxis(
                                ap=idx[:, d_out:d_out + 1], axis=0,
                            ),
                        )
                        add_dep_helper(gi.ins, fence.ins,
                                       reason="gather waits on stage fence")
                    # T1: swap (bc%32) <-> w per 32x32 square (in-place grid)
                    t1o = bigC.tile([BC, NDC * H, W], F32, tag="T1", bufs=2)
                    nc.vector.transpose(
                        t1o[:].rearrange("p a b -> p (a b)"),
                        g[:].rearrange("p a h w -> p (a h w)"),
                    )
                    # 32 permutation matmuls: roll w (on partitions)
                    w2 = bigC.tile([BC, NDC * H, W], F32, tag="W2", bufs=2)
                    for rq in range(D // 4):
                        pst = psum_pool.tile([BC, 4, NDC * H], F32,
                                             tag="pmm")
                        for jj in range(4):
                            r = rq * 4 + jj
                            nc.tensor.matmul(
                                pst[:, jj, :],
                                lhsT=sw[:, r, :],
                                rhs=t1o[:, :, r],
                                start=True, stop=True,
                            )
                        nc.scalar.copy(
                            w2[:, :, rq * 4:(rq + 1) * 4],
                            pst[:].rearrange("p j n -> p n j"),
                        )
                    # T2: swap w <-> (bc%32) back; final (d, h, w) layout
                    fin = bigC.tile([BC, NDC * H * W], F32, tag="FIN", bufs=2)
                    nc.vector.transpose(
                        fin[:],
                        w2[:].rearrange("p a b -> p (a b)"),
                    )
                    nc.sync.dma_start(
                        out[:, ch * NDC * H * W:(ch + 1) * NDC * H * W],
                        fin[:],
                    )

    nc.compile()
    return nc


_NC_CACHE = None


def _get_nc():
    global _NC_CACHE
    if _NC_CACHE is None:
        _NC_CACHE = build_kernel()
    return _NC_CACHE


def _shard(inputs):
    k = np.ascontiguousarray(np.asarray(inputs["kernel"], dtype=np.float32))
    B, C, d, h, w = k.shape
    flat = k.reshape(B * C, d * h * w)
    per = flat.shape[0] // 8
    return k.shape, [{"x": flat[i * per:(i + 1) * per]} for i in range(8)]


def kernel(**inputs: np.ndarray) -> np.ndarray:
    from concourse.bass_utils import run_bass_kernel_spmd

    shape, in_maps = _shard(inputs)
    nc = _get_nc()
    res = run_bass_kernel_spmd(nc, in_maps, core_ids=list(range(8)))
    outs = [res.results[i]["out"] for i in range(8)]
    return np.concatenate(outs, axis=0).reshape(shape)


def profile_once(inputs):
    """Run once with NTFF tracing; return exec_time_ns or None."""
    from concourse.bass_utils import run_bass_kernel_spmd

    _, in_maps = _shard(inputs)
    try:
        res = run_bass_kernel_spmd(
            _get_nc(), in_maps, core_ids=list(range(8)), trace=True
        )
        return res.exec_time_ns
    except Exception as e:
        print(f"profile_once failed: {type(e).__name__}: {e}")
        return None


def _np_reference(xs):
    v = xs.reshape(BC, D, H, W).astype(np.float64)
    ks = v.sum(axis=(1, 2, 3)) + EPS
    z = np.arange(D)
    cz = np.einsum("pdhw,d->p", v, z) / ks
    cy = np.einsum("pdhw,h->p", v, z) / ks
    cx = np.einsum("pdhw,w->p", v, z) / ks
    sz, sy, sx = (np.round(D / 2 - c).astype(int) for c in (cz, cy, cx))
    exp = np.empty_like(v, dtype=np.float32)
    for p in range(BC):
        exp[p] = np.roll(
            v[p].astype(np.float32), (sz[p], sy[p], sx[p]), axis=(0, 1, 2)
        )
    return exp.reshape(BC, VOL)


if __name__ == "__main__":
    from concourse.bass_interp import CoreSim

    rng = np.random.default_rng(0)
    xs = rng.random((BC, VOL), dtype=np.float32)

    nc = build_kernel()
    sim = CoreSim(nc, trace=False)
    sim.tensor("x")[:] = xs
    sim.simulate()
    got = np.array(sim.tensor("out"))

    exp = _np_reference(xs)
    err = np.abs(got - exp)
    rel = np.linalg.norm(got - exp) / np.linalg.norm(exp)
    badvol = int((err.reshape(BC, -1).max(1) > 1e-5).sum())
    print("cost-model time:", sim._sim_state.time, "ns")
    print("max abs err:", err.max(), "rel:", rel)
    print("mismatched volumes (rounding-boundary flips are OK in sim):",
          badvol, "/", BC)
    assert badvol <= 3, "sim mismatch beyond rounding-boundary noise"
    print("CoreSim PASS")
